# revision 3
# baseline (speedup 1.0000x reference)
"""Trainium2 Bass kernel for nn_Coefficients: assemble the sparse circuit
coefficient matrix

    out = [ kcl  = [ M | 0 ]                       (N rows)
            kvl  = [ 0 | I_E | -M^T ]              (E rows)
            elem = diag(z) / diag(y) scatter ]     (E rows)

Sharded row-wise across 8 NeuronCores: core d assembles kcl rows of
M[d*256:(d+1)*256], kvl rows of elems d*512:(d+1)*512 (incl. -M^T columns),
and elem rows d*512:(d+1)*512. The output buffer arrives pre-zeroed (the
runner donates zero-initialized buffers — unwritten elements read back as 0),
so the kernel writes only the nonzero regions: the M block (DRAM->DRAM DMA),
the -M^T block (PE transpose against -I), and the three diagonals
(indirect-DMA scatter with host-computed flat offsets).
"""

import numpy as np

N = 2048
E = 4096
W = 2 * E + N  # 10240
D = 8
NR = N // D  # 256 kcl rows per core
EC = E // D  # 512 kvl/elem rows per core
ROWS = NR + EC + EC  # 1280 output rows per core

_CACHE: dict = {}


def _build():
    import concourse.bacc as bacc
    import concourse.tile as tile
    import concourse.mybir as mybir
    import concourse.bass as bass
    from concourse._compat import get_trn_type

    f32 = mybir.dt.float32
    i32 = mybir.dt.int32

    nc = bacc.Bacc(
        get_trn_type() or "TRN2",
        target_bir_lowering=False,
        debug=False,
        enable_asserts=False,
        num_devices=D,
    )

    m_rows = nc.dram_tensor("m_rows", [NR, E], f32, kind="ExternalInput")
    m_cols = nc.dram_tensor("m_cols", [N, EC], f32, kind="ExternalInput")
    params_s = nc.dram_tensor("params_s", [128, 4], f32, kind="ExternalInput")
    kinds_s = nc.dram_tensor("kinds_s", [128, 4], i32, kind="ExternalInput")
    ei = nc.dram_tensor("ei", [128, 4], i32, kind="ExternalInput")
    zi = nc.dram_tensor("zi", [128, 4], i32, kind="ExternalInput")
    yi = nc.dram_tensor("yi", [128, 4], i32, kind="ExternalInput")
    out = nc.dram_tensor("out", [ROWS * W], f32, kind="ExternalOutput")

    out2d = out.ap().rearrange("(r w) -> r w", w=W)
    outflat = out.ap().rearrange("(n o) -> n o", o=1)

    AO = mybir.AluOpType

    with tile.TileContext(nc) as tc:
        with (
            tc.tile_pool(name="cpool", bufs=1) as cpool,
            tc.tile_pool(name="tpool", bufs=2) as tpool,
            tc.tile_pool(name="ppool", bufs=4, space="PSUM") as ppool,
        ):
            # ---- kcl M block: DRAM -> DRAM, no SBUF round trip
            nc.sync.dma_start(out=out2d[0:NR, 0:E], in_=m_rows.ap()[:, :])

            # ---- load M column shard as [n-in-chunk=128, (nchunk, e)]
            mc = cpool.tile([128, 16 * EC], f32)
            nc.scalar.dma_start(
                out=mc[:].rearrange("p (n e) -> p n e", n=16),
                in_=m_cols.ap().rearrange("(n p) e -> p n e", p=128),
            )

            # ---- identity for the PE transpose (negation happens on the
            # PSUM->SBUF copy; the is_transpose datapath ignores identity values)
            ident = cpool.tile([128, 128], f32)
            nc.gpsimd.memset(ident[:], 0.0)
            nc.gpsimd.affine_select(
                out=ident[:],
                in_=ident[:],
                compare_op=AO.not_equal,
                fill=1.0,
                base=0,
                pattern=[[-1, 128]],  # iota = p - col; !=0 keeps 0, ==0 fills 1
                channel_multiplier=1,
            )

            # ---- kvl -M^T block: 4 e-chunks of 128 rows
            for ec in range(4):
                T = tpool.tile([128, N], f32, tag="T")
                for nb in range(4):
                    ps = ppool.tile([128, 512], f32)
                    for nn in range(4):
                        nchunk = nb * 4 + nn
                        base = nchunk * EC + ec * 128
                        nc.tensor.transpose(
                            out=ps[:, nn * 128 : (nn + 1) * 128],
                            in_=mc[:, base : base + 128],
                            identity=ident[:],
                        )
                    nc.vector.tensor_scalar(
                        T[:, nb * 512 : (nb + 1) * 512], ps[:], -1.0, None, op0=AO.mult
                    )
                eng = nc.sync if ec % 2 == 0 else nc.scalar
                eng.dma_start(
                    out=out2d[NR + ec * 128 : NR + (ec + 1) * 128, 2 * E : 2 * E + N],
                    in_=T[:],
                )

            # ---- z/y diagonal values from params/kinds (layout r = c*128 + p)
            pt = cpool.tile([128, 4], f32)
            kt = cpool.tile([128, 4], f32)
            nc.sync.dma_start(out=pt[:], in_=params_s.ap()[:, :])
            nc.gpsimd.dma_start(out=kt[:], in_=kinds_s.ap()[:, :])  # i32 -> f32 cast

            rm = cpool.tile([128, 4], f32)
            im = cpool.tile([128, 4], f32)
            vm = cpool.tile([128, 4], f32)
            sm = cpool.tile([128, 4], f32)
            onm = cpool.tile([128, 4], f32)
            offm = cpool.tile([128, 4], f32)
            zv = cpool.tile([128, 4], f32)
            yv = cpool.tile([128, 4], f32)
            t0 = cpool.tile([128, 4], f32)
            t1 = cpool.tile([128, 4], f32)

            nc.vector.tensor_scalar(rm[:], kt[:], 0.0, None, op0=AO.is_equal)
            nc.vector.tensor_scalar(im[:], kt[:], 1.0, None, op0=AO.is_equal)
            nc.vector.tensor_scalar(vm[:], kt[:], 2.0, None, op0=AO.is_equal)
            nc.vector.tensor_scalar(sm[:], kt[:], 3.0, None, op0=AO.is_equal)
            nc.vector.tensor_scalar(onm[:], pt[:], 0.0, None, op0=AO.is_gt)
            nc.vector.tensor_scalar(offm[:], pt[:], 0.0, None, op0=AO.is_le)

            # z = vc + sw*off - r*params
            nc.vector.tensor_tensor(t0[:], sm[:], offm[:], op=AO.mult)
            nc.vector.tensor_tensor(t0[:], vm[:], t0[:], op=AO.add)
            nc.vector.tensor_tensor(t1[:], rm[:], pt[:], op=AO.mult)
            nc.vector.tensor_tensor(zv[:], t0[:], t1[:], op=AO.subtract)
            # y = r + ivs + sw*on
            nc.vector.tensor_tensor(t0[:], sm[:], onm[:], op=AO.mult)
            nc.vector.tensor_tensor(t0[:], im[:], t0[:], op=AO.add)
            nc.vector.tensor_tensor(yv[:], rm[:], t0[:], op=AO.add)

            ones = cpool.tile([128, 1], f32)
            nc.gpsimd.memset(ones[:], 1.0)

            # ---- index tiles + 12 indirect scatters ([128,1] each)
            eit = cpool.tile([128, 4], i32)
            zit = cpool.tile([128, 4], i32)
            yit = cpool.tile([128, 4], i32)
            nc.sync.dma_start(out=eit[:], in_=ei.ap()[:, :])
            nc.sync.dma_start(out=zit[:], in_=zi.ap()[:, :])
            nc.sync.dma_start(out=yit[:], in_=yi.ap()[:, :])

            for c in range(4):
                nc.gpsimd.indirect_dma_start(
                    out=outflat[:, :],
                    out_offset=bass.IndirectOffsetOnAxis(ap=eit[:, c : c + 1], axis=0),
                    in_=ones[:, 0:1],
                    in_offset=None,
                )
                nc.gpsimd.indirect_dma_start(
                    out=outflat[:, :],
                    out_offset=bass.IndirectOffsetOnAxis(ap=zit[:, c : c + 1], axis=0),
                    in_=zv[:, c : c + 1],
                    in_offset=None,
                )
                nc.gpsimd.indirect_dma_start(
                    out=outflat[:, :],
                    out_offset=bass.IndirectOffsetOnAxis(ap=yit[:, c : c + 1], axis=0),
                    in_=yv[:, c : c + 1],
                    in_offset=None,
                )

    nc.compile()
    return nc


def _get_nc():
    if "nc" not in _CACHE:
        _CACHE["nc"] = _build()
    return _CACHE["nc"]


def _in_maps(M, params, kinds):
    maps = []
    p = np.arange(128)[:, None]
    c = np.arange(4)[None, :]
    r = c * 128 + p  # [128, 4] local elem index
    for d in range(D):
        ei = ((NR + r) * W + (E + d * EC + r)).astype(np.int32)
        zi = ((NR + EC + r) * W + (d * EC + r)).astype(np.int32)
        yi = ((NR + EC + r) * W + (E + d * EC + r)).astype(np.int32)
        maps.append(
            {
                "m_rows": np.ascontiguousarray(M[d * NR : (d + 1) * NR, :]),
                "m_cols": np.ascontiguousarray(M[:, d * EC : (d + 1) * EC]),
                "params_s": np.ascontiguousarray(
                    params[d * EC : (d + 1) * EC].reshape(4, 128).T
                ),
                "kinds_s": np.ascontiguousarray(
                    kinds[d * EC : (d + 1) * EC].reshape(4, 128).T
                ),
                "ei": np.ascontiguousarray(ei),
                "zi": np.ascontiguousarray(zi),
                "yi": np.ascontiguousarray(yi),
            }
        )
    return maps


def kernel(M, params, kinds, _trace=False, _trace_kwargs=None):
    from concourse.bass_utils import run_bass_kernel_spmd

    M = np.ascontiguousarray(np.asarray(M, dtype=np.float32))
    params = np.ascontiguousarray(np.asarray(params, dtype=np.float32))
    kinds = np.ascontiguousarray(np.asarray(kinds, dtype=np.int32))
    assert M.shape == (N, E) and params.shape == (E,) and kinds.shape == (E,)

    nc = _get_nc()
    res = run_bass_kernel_spmd(
        nc,
        _in_maps(M, params, kinds),
        core_ids=list(range(D)),
        trace=_trace,
        **(_trace_kwargs or {}),
    )
    parts = [res.results[d]["out"].reshape(ROWS, W) for d in range(D)]
    full = np.concatenate(
        [q[0:NR] for q in parts]
        + [q[NR : NR + EC] for q in parts]
        + [q[NR + EC : ROWS] for q in parts],
        axis=0,
    )
    if _trace:
        _CACHE["last_result"] = res
    return full


# revision 4
# speedup vs baseline: 1.4431x; 1.4431x over previous
"""Trainium2 Bass kernel for nn_Coefficients: assemble the sparse circuit
coefficient matrix

    out = [ kcl  = [ M | 0 ]                       (N rows)
            kvl  = [ 0 | I_E | -M^T ]              (E rows)
            elem = diag(z) / diag(y) scatter ]     (E rows)

Row-wise shard across 8 NeuronCores: core d produces
  - kcl:  M[d*256:(d+1)*256, :]            (DRAM->DRAM copy)
  - mt:   -M[:, d*512:(d+1)*512]^T         (PE transpose + negate)
  - eye:  I bands (512x128), zb/yb: diag(z)/diag(y) bands computed from
          params/kinds on device.
The host unshards: places each core's blocks/bands at their row/column
offsets in the zero canvas (pure indexing — all numeric content is
device-produced).

The m_cols load trick: a flat [2048,512] DRAM block reshaped to SBUF
[128, 2048] quarters keeps every DMA descriptor 8KB-contiguous; the
resulting n = 16*p + 4*jg + jj interleave is undone for free in the
PSUM->SBUF copy's strided access pattern.
"""

import numpy as np

N = 2048
E = 4096
W = 2 * E + N  # 10240
D = 8
NR = N // D  # 256 kcl rows per core
EC = E // D  # 512 kvl/elem rows per core

_CACHE: dict = {}


def _build():
    import concourse.bacc as bacc
    import concourse.tile as tile
    import concourse.mybir as mybir
    from concourse._compat import get_trn_type

    f32 = mybir.dt.float32
    i32 = mybir.dt.int32

    nc = bacc.Bacc(
        get_trn_type() or "TRN2",
        target_bir_lowering=False,
        debug=False,
        enable_asserts=False,
        num_devices=D,
    )

    m_rows = nc.dram_tensor("m_rows", [NR, E], f32, kind="ExternalInput")
    m_cols = nc.dram_tensor("m_cols", [N, EC], f32, kind="ExternalInput")
    params_s = nc.dram_tensor("params_s", [128, 4], f32, kind="ExternalInput")
    kinds_s = nc.dram_tensor("kinds_s", [128, 4], i32, kind="ExternalInput")

    kcl = nc.dram_tensor("kcl", [NR, E], f32, kind="ExternalOutput")
    mt = nc.dram_tensor("mt", [EC, N], f32, kind="ExternalOutput")
    eye = nc.dram_tensor("eye", [EC, 128], f32, kind="ExternalOutput")
    zb = nc.dram_tensor("zb", [EC, 128], f32, kind="ExternalOutput")
    yb = nc.dram_tensor("yb", [EC, 128], f32, kind="ExternalOutput")

    AO = mybir.AluOpType

    # m_cols flat view: element (n, e) lives at flat n*512+e; SBUF quarter jg
    # holds partitions p with contiguous 8KB runs: n = 16p + 4*jg + jj.
    mflat = m_cols.ap().rearrange("n e -> (n e)").rearrange(
        "(p q f) -> p q f", p=128, q=4
    )  # [p, jg, 2048] with per-(p,jg) contiguous 2048 f32

    with tile.TileContext(nc) as tc:
        with (
            tc.tile_pool(name="cpool", bufs=1) as cpool,
            tc.tile_pool(name="tpool", bufs=2) as tpool,
            tc.tile_pool(name="ppool", bufs=4, space="PSUM") as ppool,
        ):
            # ---- kcl M block: DRAM -> DRAM (both sides fully contiguous)
            nc.sync.dma_start(out=kcl.ap()[:, :], in_=m_rows.ap()[:, :])

            # ---- m_cols quarters: [128, 2048], 8KB contiguous per partition
            mcq = []
            for jg in range(4):
                t = cpool.tile([128, 2048], f32, tag=f"mc{jg}")
                eng = nc.scalar if jg % 2 == 0 else nc.sync
                eng.dma_start(out=t[:], in_=mflat[:, jg, :])
                mcq.append(t)

            # ---- small inputs
            pt = cpool.tile([128, 4], f32)
            kt = cpool.tile([128, 4], f32)
            nc.sync.dma_start(out=pt[:], in_=params_s.ap()[:, :])
            nc.gpsimd.dma_start(out=kt[:], in_=kinds_s.ap()[:, :])  # i32 -> f32

            # ---- identity tile (also the eye-band payload)
            ident = cpool.tile([128, 128], f32)
            nc.gpsimd.memset(ident[:], 0.0)
            nc.gpsimd.affine_select(
                out=ident[:],
                in_=ident[:],
                compare_op=AO.not_equal,
                fill=1.0,
                base=0,
                pattern=[[-1, 128]],
                channel_multiplier=1,
            )

            # ---- z/y diagonal values (layout r = c*128 + p)
            rm = cpool.tile([128, 4], f32)
            im = cpool.tile([128, 4], f32)
            vm = cpool.tile([128, 4], f32)
            sm = cpool.tile([128, 4], f32)
            onm = cpool.tile([128, 4], f32)
            offm = cpool.tile([128, 4], f32)
            zv = cpool.tile([128, 4], f32)
            yv = cpool.tile([128, 4], f32)
            t0 = cpool.tile([128, 4], f32)
            t1 = cpool.tile([128, 4], f32)

            nc.vector.tensor_scalar(rm[:], kt[:], 0.0, None, op0=AO.is_equal)
            nc.vector.tensor_scalar(im[:], kt[:], 1.0, None, op0=AO.is_equal)
            nc.vector.tensor_scalar(vm[:], kt[:], 2.0, None, op0=AO.is_equal)
            nc.vector.tensor_scalar(sm[:], kt[:], 3.0, None, op0=AO.is_equal)
            nc.vector.tensor_scalar(onm[:], pt[:], 0.0, None, op0=AO.is_gt)
            nc.vector.tensor_scalar(offm[:], pt[:], 0.0, None, op0=AO.is_le)
            # z = vc + sw*off - r*params
            nc.vector.tensor_tensor(t0[:], sm[:], offm[:], op=AO.mult)
            nc.vector.tensor_tensor(t0[:], vm[:], t0[:], op=AO.add)
            nc.vector.tensor_tensor(t1[:], rm[:], pt[:], op=AO.mult)
            nc.vector.tensor_tensor(zv[:], t0[:], t1[:], op=AO.subtract)
            # y = r + ivs + sw*on
            nc.vector.tensor_tensor(t0[:], sm[:], onm[:], op=AO.mult)
            nc.vector.tensor_tensor(t0[:], im[:], t0[:], op=AO.add)
            nc.vector.tensor_tensor(yv[:], rm[:], t0[:], op=AO.add)

            # ---- diagonal bands out (gpsimd queue; tiny, fully overlapped)
            for c in range(4):
                zd = tpool.tile([128, 128], f32, tag="zd")
                yd = tpool.tile([128, 128], f32, tag="yd")
                nc.vector.tensor_scalar(zd[:], ident[:], zv[:, c : c + 1], None, op0=AO.mult)
                nc.vector.tensor_scalar(yd[:], ident[:], yv[:, c : c + 1], None, op0=AO.mult)
                nc.gpsimd.dma_start(out=eye.ap()[c * 128 : (c + 1) * 128, :], in_=ident[:])
                nc.gpsimd.dma_start(out=zb.ap()[c * 128 : (c + 1) * 128, :], in_=zd[:])
                nc.gpsimd.dma_start(out=yb.ap()[c * 128 : (c + 1) * 128, :], in_=yd[:])

            # ---- -M^T: PE transpose, n = 16p + 4jg + jj undone in copy APs
            for ec in range(4):
                T = tpool.tile([128, N], f32, tag="T")
                # dst view [e, j(16), p2(128)]: free index = p2*16 + j
                Tv = T[:].rearrange("e (p2 j) -> e j p2", j=16)
                for jg in range(4):
                    ps = ppool.tile([128, 512], f32)
                    for jj in range(4):
                        nc.tensor.transpose(
                            out=ps[:, jj * 128 : (jj + 1) * 128],
                            in_=mcq[jg][:, jj * 512 + ec * 128 : jj * 512 + ec * 128 + 128],
                            identity=ident[:],
                        )
                    # negate + un-interleave: T[e, 16*p2 + 4*jg + jj] = -ps[e, jj*128+p2]
                    nc.vector.tensor_scalar(
                        Tv[:, 4 * jg : 4 * jg + 4, :],
                        ps[:].rearrange("e (jj p2) -> e jj p2", p2=128),
                        -1.0,
                        None,
                        op0=AO.mult,
                    )
                eng = nc.sync if ec % 2 == 0 else nc.scalar
                eng.dma_start(out=mt.ap()[ec * 128 : (ec + 1) * 128, :], in_=T[:])

    nc.compile()
    return nc


def _get_nc():
    if "nc" not in _CACHE:
        _CACHE["nc"] = _build()
    return _CACHE["nc"]


def _in_maps(M, params, kinds):
    maps = []
    for d in range(D):
        maps.append(
            {
                "m_rows": np.ascontiguousarray(M[d * NR : (d + 1) * NR, :]),
                "m_cols": np.ascontiguousarray(M[:, d * EC : (d + 1) * EC]),
                "params_s": np.ascontiguousarray(
                    params[d * EC : (d + 1) * EC].reshape(4, 128).T
                ),
                "kinds_s": np.ascontiguousarray(
                    kinds[d * EC : (d + 1) * EC].reshape(4, 128).T
                ),
            }
        )
    return maps


def kernel(M, params, kinds, _trace=False, _trace_kwargs=None):
    from concourse.bass_utils import run_bass_kernel_spmd

    M = np.ascontiguousarray(np.asarray(M, dtype=np.float32))
    params = np.ascontiguousarray(np.asarray(params, dtype=np.float32))
    kinds = np.ascontiguousarray(np.asarray(kinds, dtype=np.int32))
    assert M.shape == (N, E) and params.shape == (E,) and kinds.shape == (E,)

    nc = _get_nc()
    res = run_bass_kernel_spmd(
        nc,
        _in_maps(M, params, kinds),
        core_ids=list(range(D)),
        trace=_trace,
        **(_trace_kwargs or {}),
    )
    out = np.zeros((N + 2 * E, W), np.float32)
    for d in range(D):
        r = res.results[d]
        out[d * NR : (d + 1) * NR, 0:E] = r["kcl"]
        out[N + d * EC : N + (d + 1) * EC, 2 * E :] = r["mt"]
        for c in range(4):
            rb = c * 128
            g0 = d * EC + c * 128  # global elem index of band start
            out[N + g0 : N + g0 + 128, E + g0 : E + g0 + 128] = r["eye"][rb : rb + 128]
            out[N + E + g0 : N + E + g0 + 128, g0 : g0 + 128] = r["zb"][rb : rb + 128]
            out[N + E + g0 : N + E + g0 + 128, E + g0 : E + g0 + 128] = r["yb"][
                rb : rb + 128
            ]
    if _trace:
        _CACHE["last_result"] = res
    return out


# revision 5
# speedup vs baseline: 1.5464x; 1.0716x over previous
"""Trainium2 Bass kernel for nn_Coefficients: assemble the sparse circuit
coefficient matrix

    out = [ kcl  = [ M | 0 ]                       (N rows)
            kvl  = [ 0 | I_E | -M^T ]              (E rows)
            elem = diag(z) / diag(y) scatter ]     (E rows)

Row-wise shard across 8 NeuronCores: core d produces
  - kcl:  M[d*256:(d+1)*256, :]            (DRAM->DRAM copy)
  - mt:   -M[:, d*512:(d+1)*512]^T         (PE transpose + negate)
  - eye:  I bands (512x128), zb/yb: diag(z)/diag(y) bands computed from
          params/kinds on device.
The host unshards: places each core's blocks/bands at their row/column
offsets in the zero canvas (pure indexing — all numeric content is
device-produced).

The m_cols load trick: a flat [2048,512] DRAM block reshaped to SBUF
[128, 2048] quarters keeps every DMA descriptor 8KB-contiguous; the
resulting n = 16*p + 4*jg + jj interleave is undone for free in the
PSUM->SBUF copy's strided access pattern.
"""

import numpy as np

N = 2048
E = 4096
W = 2 * E + N  # 10240
D = 8
NR = N // D  # 256 kcl rows per core
EC = E // D  # 512 kvl/elem rows per core

_CACHE: dict = {}


def _build():
    import concourse.bacc as bacc
    import concourse.tile as tile
    import concourse.mybir as mybir
    from concourse._compat import get_trn_type

    f32 = mybir.dt.float32
    i32 = mybir.dt.int32

    nc = bacc.Bacc(
        get_trn_type() or "TRN2",
        target_bir_lowering=False,
        debug=False,
        enable_asserts=False,
        num_devices=D,
    )

    m_rows = nc.dram_tensor("m_rows", [NR, E], f32, kind="ExternalInput")
    m_cols = nc.dram_tensor("m_cols", [N, EC], f32, kind="ExternalInput")
    params_s = nc.dram_tensor("params_s", [128, 4], f32, kind="ExternalInput")
    kinds_s = nc.dram_tensor("kinds_s", [128, 4], i32, kind="ExternalInput")

    kcl = nc.dram_tensor("kcl", [NR, E], f32, kind="ExternalOutput")
    mt = nc.dram_tensor("mt", [EC, N], f32, kind="ExternalOutput")
    eye = nc.dram_tensor("eye", [EC, 128], f32, kind="ExternalOutput")
    zb = nc.dram_tensor("zb", [EC, 128], f32, kind="ExternalOutput")
    yb = nc.dram_tensor("yb", [EC, 128], f32, kind="ExternalOutput")

    AO = mybir.AluOpType

    # m_cols flat view: element (n, e) lives at flat n*512+e; SBUF quarter jg
    # holds partitions p with contiguous 8KB runs: n = 16p + 4*jg + jj.
    mflat = m_cols.ap().rearrange("n e -> (n e)").rearrange(
        "(p q f) -> p q f", p=128, q=4
    )  # [p, jg, 2048] with per-(p,jg) contiguous 2048 f32

    with tile.TileContext(nc) as tc:
        with (
            tc.tile_pool(name="cpool", bufs=1) as cpool,
            tc.tile_pool(name="tpool", bufs=2) as tpool,
            tc.tile_pool(name="ppool", bufs=4, space="PSUM") as ppool,
        ):
            # ---- m_cols quarters first on both HWDGE rings so the PE can
            # start ASAP: [128, 2048], 8KB contiguous per partition
            mcq = []
            for jg in range(4):
                t = cpool.tile([128, 2048], f32, tag=f"mc{jg}")
                eng = nc.sync if jg % 2 == 0 else nc.scalar
                eng.dma_start(out=t[:], in_=mflat[:, jg, :])
                mcq.append(t)

            # ---- small inputs
            pt = cpool.tile([128, 4], f32)
            kt = cpool.tile([128, 4], f32)
            nc.sync.dma_start(out=pt[:], in_=params_s.ap()[:, :])
            nc.gpsimd.dma_start(out=kt[:], in_=kinds_s.ap()[:, :])  # i32 -> f32

            # ---- kcl M block: DRAM -> DRAM, no dependents; queued behind the
            # mc loads (one half per ring) so it drains during the PE phase
            nc.sync.dma_start(
                out=kcl.ap()[0 : NR // 2, :], in_=m_rows.ap()[0 : NR // 2, :]
            )
            nc.scalar.dma_start(
                out=kcl.ap()[NR // 2 : NR, :], in_=m_rows.ap()[NR // 2 : NR, :]
            )

            # ---- identity tile (also the eye-band payload)
            ident = cpool.tile([128, 128], f32)
            nc.gpsimd.memset(ident[:], 0.0)
            nc.gpsimd.affine_select(
                out=ident[:],
                in_=ident[:],
                compare_op=AO.not_equal,
                fill=1.0,
                base=0,
                pattern=[[-1, 128]],
                channel_multiplier=1,
            )

            # ---- z/y diagonal values (layout r = c*128 + p)
            rm = cpool.tile([128, 4], f32)
            im = cpool.tile([128, 4], f32)
            vm = cpool.tile([128, 4], f32)
            sm = cpool.tile([128, 4], f32)
            onm = cpool.tile([128, 4], f32)
            offm = cpool.tile([128, 4], f32)
            zv = cpool.tile([128, 4], f32)
            yv = cpool.tile([128, 4], f32)
            t0 = cpool.tile([128, 4], f32)
            t1 = cpool.tile([128, 4], f32)

            nc.vector.tensor_scalar(rm[:], kt[:], 0.0, None, op0=AO.is_equal)
            nc.vector.tensor_scalar(im[:], kt[:], 1.0, None, op0=AO.is_equal)
            nc.vector.tensor_scalar(vm[:], kt[:], 2.0, None, op0=AO.is_equal)
            nc.vector.tensor_scalar(sm[:], kt[:], 3.0, None, op0=AO.is_equal)
            nc.vector.tensor_scalar(onm[:], pt[:], 0.0, None, op0=AO.is_gt)
            nc.vector.tensor_scalar(offm[:], pt[:], 0.0, None, op0=AO.is_le)
            # z = vc + sw*off - r*params
            nc.vector.tensor_tensor(t0[:], sm[:], offm[:], op=AO.mult)
            nc.vector.tensor_tensor(t0[:], vm[:], t0[:], op=AO.add)
            nc.vector.tensor_tensor(t1[:], rm[:], pt[:], op=AO.mult)
            nc.vector.tensor_tensor(zv[:], t0[:], t1[:], op=AO.subtract)
            # y = r + ivs + sw*on
            nc.vector.tensor_tensor(t0[:], sm[:], onm[:], op=AO.mult)
            nc.vector.tensor_tensor(t0[:], im[:], t0[:], op=AO.add)
            nc.vector.tensor_tensor(yv[:], rm[:], t0[:], op=AO.add)

            # ---- diagonal bands out (gpsimd queue; tiny, fully overlapped)
            for c in range(4):
                zd = tpool.tile([128, 128], f32, tag="zd")
                yd = tpool.tile([128, 128], f32, tag="yd")
                nc.vector.tensor_scalar(zd[:], ident[:], zv[:, c : c + 1], None, op0=AO.mult)
                nc.vector.tensor_scalar(yd[:], ident[:], yv[:, c : c + 1], None, op0=AO.mult)
                nc.gpsimd.dma_start(out=eye.ap()[c * 128 : (c + 1) * 128, :], in_=ident[:])
                nc.gpsimd.dma_start(out=zb.ap()[c * 128 : (c + 1) * 128, :], in_=zd[:])
                nc.gpsimd.dma_start(out=yb.ap()[c * 128 : (c + 1) * 128, :], in_=yd[:])

            # ---- -M^T: PE transpose, n = 16p + 4jg + jj undone in copy APs
            for ec in range(4):
                T = tpool.tile([128, N], f32, tag="T")
                # dst view [e, j(16), p2(128)]: free index = p2*16 + j
                Tv = T[:].rearrange("e (p2 j) -> e j p2", j=16)
                for jg in range(4):
                    ps = ppool.tile([128, 512], f32)
                    for jj in range(4):
                        nc.tensor.transpose(
                            out=ps[:, jj * 128 : (jj + 1) * 128],
                            in_=mcq[jg][:, jj * 512 + ec * 128 : jj * 512 + ec * 128 + 128],
                            identity=ident[:],
                        )
                    # negate + un-interleave: T[e, 16*p2 + 4*jg + jj] = -ps[e, jj*128+p2]
                    nc.vector.tensor_scalar(
                        Tv[:, 4 * jg : 4 * jg + 4, :],
                        ps[:].rearrange("e (jj p2) -> e jj p2", p2=128),
                        -1.0,
                        None,
                        op0=AO.mult,
                    )
                eng = nc.sync if ec % 2 == 0 else nc.scalar
                eng.dma_start(out=mt.ap()[ec * 128 : (ec + 1) * 128, :], in_=T[:])

    nc.compile()
    return nc


def _get_nc():
    if "nc" not in _CACHE:
        _CACHE["nc"] = _build()
    return _CACHE["nc"]


def _in_maps(M, params, kinds):
    maps = []
    for d in range(D):
        maps.append(
            {
                "m_rows": np.ascontiguousarray(M[d * NR : (d + 1) * NR, :]),
                "m_cols": np.ascontiguousarray(M[:, d * EC : (d + 1) * EC]),
                "params_s": np.ascontiguousarray(
                    params[d * EC : (d + 1) * EC].reshape(4, 128).T
                ),
                "kinds_s": np.ascontiguousarray(
                    kinds[d * EC : (d + 1) * EC].reshape(4, 128).T
                ),
            }
        )
    return maps


def kernel(M, params, kinds, _trace=False, _trace_kwargs=None):
    from concourse.bass_utils import run_bass_kernel_spmd

    M = np.ascontiguousarray(np.asarray(M, dtype=np.float32))
    params = np.ascontiguousarray(np.asarray(params, dtype=np.float32))
    kinds = np.ascontiguousarray(np.asarray(kinds, dtype=np.int32))
    assert M.shape == (N, E) and params.shape == (E,) and kinds.shape == (E,)

    nc = _get_nc()
    res = run_bass_kernel_spmd(
        nc,
        _in_maps(M, params, kinds),
        core_ids=list(range(D)),
        trace=_trace,
        **(_trace_kwargs or {}),
    )
    out = np.zeros((N + 2 * E, W), np.float32)
    for d in range(D):
        r = res.results[d]
        out[d * NR : (d + 1) * NR, 0:E] = r["kcl"]
        out[N + d * EC : N + (d + 1) * EC, 2 * E :] = r["mt"]
        for c in range(4):
            rb = c * 128
            g0 = d * EC + c * 128  # global elem index of band start
            out[N + g0 : N + g0 + 128, E + g0 : E + g0 + 128] = r["eye"][rb : rb + 128]
            out[N + E + g0 : N + E + g0 + 128, g0 : g0 + 128] = r["zb"][rb : rb + 128]
            out[N + E + g0 : N + E + g0 + 128, E + g0 : E + g0 + 128] = r["yb"][
                rb : rb + 128
            ]
    if _trace:
        _CACHE["last_result"] = res
    return out


# revision 7
# speedup vs baseline: 1.8762x; 1.2133x over previous
"""Trainium2 Bass kernel for nn_Coefficients: assemble the sparse circuit
coefficient matrix

    out = [ kcl  = [ M | 0 ]                       (N rows)
            kvl  = [ 0 | I_E | -M^T ]              (E rows)
            elem = diag(z) / diag(y) scatter ]     (E rows)

Row-wise shard across 8 NeuronCores: core d produces
  - kcl:  M[d*256:(d+1)*256, :]            (DRAM->DRAM copy)
  - mt:   -M[:, d*512:(d+1)*512]^T         (PE transpose + negate)
  - eye:  I bands (512x128), zb/yb: diag(z)/diag(y) bands computed from
          params/kinds on device.
The host unshards: places each core's blocks/bands at their row/column
offsets in the zero canvas (pure indexing — all numeric content is
device-produced).

The m_cols load trick: a flat [2048,512] DRAM block reshaped to SBUF
[128, 2048] quarters keeps every DMA descriptor 8KB-contiguous; the
resulting n = 16*p + 4*jg + jj interleave is undone for free in the
PSUM->SBUF copy's strided access pattern.
"""

import numpy as np

N = 2048
E = 4096
W = 2 * E + N  # 10240
D = 8
NR = N // D  # 256 kcl rows per core
EC = E // D  # 512 kvl/elem rows per core

_CACHE: dict = {}


def _build():
    import concourse.bacc as bacc
    import concourse.tile as tile
    import concourse.mybir as mybir
    from concourse._compat import get_trn_type

    f32 = mybir.dt.float32
    i32 = mybir.dt.int32

    nc = bacc.Bacc(
        get_trn_type() or "TRN2",
        target_bir_lowering=False,
        debug=False,
        enable_asserts=False,
        num_devices=D,
    )

    m_rows = nc.dram_tensor("m_rows", [NR, E], f32, kind="ExternalInput")
    m_cols = nc.dram_tensor("m_cols", [N, EC], f32, kind="ExternalInput")
    params_s = nc.dram_tensor("params_s", [128, 4], f32, kind="ExternalInput")
    kinds_s = nc.dram_tensor("kinds_s", [128, 4], i32, kind="ExternalInput")

    kcl = nc.dram_tensor("kcl", [NR, E], f32, kind="ExternalOutput")
    mt = nc.dram_tensor("mt", [EC, N], f32, kind="ExternalOutput")
    eye = nc.dram_tensor("eye", [EC, 128], f32, kind="ExternalOutput")
    zb = nc.dram_tensor("zb", [EC, 128], f32, kind="ExternalOutput")
    yb = nc.dram_tensor("yb", [EC, 128], f32, kind="ExternalOutput")

    AO = mybir.AluOpType

    # m_cols flat view: element (n, e) lives at flat n*512+e; SBUF quarter jg
    # holds partitions p with contiguous 8KB runs: n = 16p + 4*jg + jj.
    mflat = m_cols.ap().rearrange("n e -> (n e)").rearrange(
        "(p q f) -> p q f", p=128, q=4
    )  # [p, jg, 2048] with per-(p,jg) contiguous 2048 f32

    with tile.TileContext(nc) as tc:
        with (
            tc.tile_pool(name="cpool", bufs=1) as cpool,
            tc.tile_pool(name="tpool", bufs=2) as tpool,
            tc.tile_pool(name="ppool", bufs=8, space="PSUM") as ppool,
        ):
            # ---- m_cols quarters first on both HWDGE rings so the PE can
            # start ASAP: [128, 2048], 8KB contiguous per partition
            mcq = []
            for jg in range(4):
                t = cpool.tile([128, 2048], f32, tag=f"mc{jg}")
                eng = nc.sync if jg % 2 == 0 else nc.scalar
                eng.dma_start(out=t[:], in_=mflat[:, jg, :])
                mcq.append(t)

            # ---- small inputs
            pt = cpool.tile([128, 4], f32)
            kt = cpool.tile([128, 4], f32)
            nc.sync.dma_start(out=pt[:], in_=params_s.ap()[:, :])
            nc.gpsimd.dma_start(out=kt[:], in_=kinds_s.ap()[:, :])  # i32 -> f32

            # ---- kcl M block: DRAM -> DRAM, no dependents; queued behind the
            # mc loads (one half per ring) so it drains during the PE phase
            nc.sync.dma_start(
                out=kcl.ap()[0 : NR // 2, :], in_=m_rows.ap()[0 : NR // 2, :]
            )
            nc.scalar.dma_start(
                out=kcl.ap()[NR // 2 : NR, :], in_=m_rows.ap()[NR // 2 : NR, :]
            )

            # ---- identity tile (also the eye-band payload)
            ident = cpool.tile([128, 128], f32)
            nc.gpsimd.memset(ident[:], 0.0)
            nc.gpsimd.affine_select(
                out=ident[:],
                in_=ident[:],
                compare_op=AO.not_equal,
                fill=1.0,
                base=0,
                pattern=[[-1, 128]],
                channel_multiplier=1,
            )

            # ---- z/y diagonal values (layout r = c*128 + p)
            rm = cpool.tile([128, 4], f32)
            im = cpool.tile([128, 4], f32)
            vm = cpool.tile([128, 4], f32)
            sm = cpool.tile([128, 4], f32)
            onm = cpool.tile([128, 4], f32)
            offm = cpool.tile([128, 4], f32)
            zv = cpool.tile([128, 4], f32)
            yv = cpool.tile([128, 4], f32)
            t0 = cpool.tile([128, 4], f32)
            t1 = cpool.tile([128, 4], f32)

            nc.vector.tensor_scalar(rm[:], kt[:], 0.0, None, op0=AO.is_equal)
            nc.vector.tensor_scalar(im[:], kt[:], 1.0, None, op0=AO.is_equal)
            nc.vector.tensor_scalar(vm[:], kt[:], 2.0, None, op0=AO.is_equal)
            nc.vector.tensor_scalar(sm[:], kt[:], 3.0, None, op0=AO.is_equal)
            nc.vector.tensor_scalar(onm[:], pt[:], 0.0, None, op0=AO.is_gt)
            nc.vector.tensor_scalar(offm[:], pt[:], 0.0, None, op0=AO.is_le)
            # z = vc + sw*off - r*params
            nc.vector.tensor_tensor(t0[:], sm[:], offm[:], op=AO.mult)
            nc.vector.tensor_tensor(t0[:], vm[:], t0[:], op=AO.add)
            nc.vector.tensor_tensor(t1[:], rm[:], pt[:], op=AO.mult)
            nc.vector.tensor_tensor(zv[:], t0[:], t1[:], op=AO.subtract)
            # y = r + ivs + sw*on
            nc.vector.tensor_tensor(t0[:], sm[:], onm[:], op=AO.mult)
            nc.vector.tensor_tensor(t0[:], im[:], t0[:], op=AO.add)
            nc.vector.tensor_tensor(yv[:], rm[:], t0[:], op=AO.add)

            # ---- diagonal bands out (gpsimd queue; tiny, fully overlapped)
            for c in range(4):
                zd = tpool.tile([128, 128], f32, tag="zd")
                yd = tpool.tile([128, 128], f32, tag="yd")
                nc.vector.tensor_scalar(zd[:], ident[:], zv[:, c : c + 1], None, op0=AO.mult)
                nc.vector.tensor_scalar(yd[:], ident[:], yv[:, c : c + 1], None, op0=AO.mult)
                nc.gpsimd.dma_start(out=eye.ap()[c * 128 : (c + 1) * 128, :], in_=ident[:])
                nc.gpsimd.dma_start(out=zb.ap()[c * 128 : (c + 1) * 128, :], in_=zd[:])
                nc.gpsimd.dma_start(out=yb.ap()[c * 128 : (c + 1) * 128, :], in_=yd[:])

            # ---- -M^T: PE transpose, n = 16p + 4jg + jj undone in copy APs
            for ec in range(4):
                T = tpool.tile([128, N], f32, tag="T")
                # dst view [e, j(16), p2(128)]: free index = p2*16 + j
                Tv = T[:].rearrange("e (p2 j) -> e j p2", j=16)
                for jg in range(4):
                    ps = ppool.tile([128, 512], f32)
                    for jj in range(4):
                        nc.tensor.transpose(
                            out=ps[:, jj * 128 : (jj + 1) * 128],
                            in_=mcq[jg][:, jj * 512 + ec * 128 : jj * 512 + ec * 128 + 128],
                            identity=ident[:],
                        )
                    # negate + un-interleave: T[e, 16*p2 + 4*jg + jj] = -ps[e, jj*128+p2]
                    # alternate DVE / ACT so neither engine paces the PE
                    dst = Tv[:, 4 * jg : 4 * jg + 4, :]
                    src = ps[:].rearrange("e (jj p2) -> e jj p2", p2=128)
                    if (ec * 4 + jg) % 2 == 0:
                        nc.vector.tensor_scalar(dst, src, -1.0, None, op0=AO.mult)
                    else:
                        nc.scalar.activation(
                            dst, src, mybir.ActivationFunctionType.Copy, scale=-1.0
                        )
                eng = nc.sync if ec % 2 == 0 else nc.scalar
                eng.dma_start(out=mt.ap()[ec * 128 : (ec + 1) * 128, :], in_=T[:])

    nc.compile()
    return nc


def _get_nc():
    if "nc" not in _CACHE:
        _CACHE["nc"] = _build()
    return _CACHE["nc"]


def _in_maps(M, params, kinds):
    maps = []
    for d in range(D):
        maps.append(
            {
                "m_rows": np.ascontiguousarray(M[d * NR : (d + 1) * NR, :]),
                "m_cols": np.ascontiguousarray(M[:, d * EC : (d + 1) * EC]),
                "params_s": np.ascontiguousarray(
                    params[d * EC : (d + 1) * EC].reshape(4, 128).T
                ),
                "kinds_s": np.ascontiguousarray(
                    kinds[d * EC : (d + 1) * EC].reshape(4, 128).T
                ),
            }
        )
    return maps


def kernel(M, params, kinds, _trace=False, _trace_kwargs=None):
    from concourse.bass_utils import run_bass_kernel_spmd

    M = np.ascontiguousarray(np.asarray(M, dtype=np.float32))
    params = np.ascontiguousarray(np.asarray(params, dtype=np.float32))
    kinds = np.ascontiguousarray(np.asarray(kinds, dtype=np.int32))
    assert M.shape == (N, E) and params.shape == (E,) and kinds.shape == (E,)

    nc = _get_nc()
    res = run_bass_kernel_spmd(
        nc,
        _in_maps(M, params, kinds),
        core_ids=list(range(D)),
        trace=_trace,
        **(_trace_kwargs or {}),
    )
    out = np.zeros((N + 2 * E, W), np.float32)
    for d in range(D):
        r = res.results[d]
        out[d * NR : (d + 1) * NR, 0:E] = r["kcl"]
        out[N + d * EC : N + (d + 1) * EC, 2 * E :] = r["mt"]
        for c in range(4):
            rb = c * 128
            g0 = d * EC + c * 128  # global elem index of band start
            out[N + g0 : N + g0 + 128, E + g0 : E + g0 + 128] = r["eye"][rb : rb + 128]
            out[N + E + g0 : N + E + g0 + 128, g0 : g0 + 128] = r["zb"][rb : rb + 128]
            out[N + E + g0 : N + E + g0 + 128, E + g0 : E + g0 + 128] = r["yb"][
                rb : rb + 128
            ]
    if _trace:
        _CACHE["last_result"] = res
    return out
